# revision 39
# baseline (speedup 1.0000x reference)
"""MultiHeadAttention on 8 TRN2 NeuronCores — v5.

Sharding: core c = (batch b=c//4, head-group g=c%4). Each core computes
Q/K/V projections for its 4 heads (d_model slice 256g:256g+256) over its
batch's 2048 tokens, full attention for those heads, a partial output
projection over all 2048 batch tokens (contraction over its 256 dm rows),
then ONE 4-rank bf16 ReduceScatter (4MB/rank) hands each core its final
512-token output slice.

vs v3: no x AllGather, RS shrinks 16MB f32 8-rank -> 4MB bf16 4-rank,
all-bf16 matmuls, V projected directly in [tok, dm] layout (no PE
transposes), bk dropped (softmax is invariant to per-query score
shifts), bv folded into bo on the host (bo_eff = bo + bv @ wo).
"""

import numpy as np
import ml_dtypes

import concourse.bass as bass
import concourse.tile as tile
from concourse import bacc, mybir
from concourse.bass_utils import axon_active, run_bass_kernel_spmd

N_CORES = 8
B, S, D = 2, 2048, 1024
H = 16          # heads total
HL = 4          # heads per core
DK = 64
TB = 2048       # tokens per batch (local q/k length)
TOUT = 512      # output tokens per core
F32 = mybir.dt.float32
BF16 = mybir.dt.bfloat16
Exp = mybir.ActivationFunctionType.Exp
Identity = mybir.ActivationFunctionType.Identity
Reciprocal = mybir.ActivationFunctionType.Reciprocal
BF = ml_dtypes.bfloat16

_cache = {}


def _build(repeat=1):
    nc = bacc.Bacc("TRN2", target_bir_lowering=False, debug=False,
                   num_devices=N_CORES)
    xp_d = nc.dram_tensor("xp", [128, 8, TB], BF16, kind="ExternalInput").ap()
    wq_d = nc.dram_tensor("wqp", [128, 8, 2, 128], BF16, kind="ExternalInput").ap()
    wk_d = nc.dram_tensor("wkp", [128, 8, 2, 128], BF16, kind="ExternalInput").ap()
    wv_d = nc.dram_tensor("wvp", [128, 8, 256], BF16, kind="ExternalInput").ap()
    wo_d = nc.dram_tensor("wop", [128, 2, D], BF16, kind="ExternalInput").ap()
    bq_d = nc.dram_tensor("bqc", [128, 2], F32, kind="ExternalInput").ap()
    out_d = nc.dram_tensor("out", [TOUT, D], BF16, kind="ExternalOutput").ap()

    with tile.TileContext(nc) as tc:
        with (
            tc.tile_pool(name="dram", bufs=1, space="DRAM") as dram,
            tc.tile_pool(name="persist", bufs=1) as pp,
        ):
            part_d = dram.tile([TB, D], BF16, tag="partd")
            rs_d = dram.tile([TOUT, D], BF16, tag="rsd")

            wq_sb = pp.tile([128, 8, 2, 128], BF16, tag="wq")
            wk_sb = pp.tile([128, 8, 2, 128], BF16, tag="wk")
            wv_sb = pp.tile([128, 8, 256], BF16, tag="wv")
            wo_sb = pp.tile([128, 2, D], BF16, tag="wo")
            bq_sb = pp.tile([128, 2], F32, tag="bq")
            nc.gpsimd.dma_start(wq_sb[:], wq_d[:])
            nc.gpsimd.dma_start(wk_sb[:], wk_d[:])
            nc.gpsimd.dma_start(wv_sb[:], wv_d[:])
            nc.gpsimd.dma_start(wo_sb[:], wo_d[:])
            nc.gpsimd.dma_start(bq_sb[:], bq_d[:])

            xT = pp.tile([128, 8, TB], BF16, tag="xT")
            QT = pp.tile([128, 2, TB], BF16, tag="QT")
            KT = pp.tile([128, 2, TB], BF16, tag="KT")
            # V in natural [tok, dm] layout + ones column for the softmax
            # denominator: lhsT chunks [128 tok, 64 V | 1].
            Vb = pp.tile([128, 16, HL, 65], BF16, tag="vb")
            nc.vector.memset(Vb[:, :, :, 64:65], 1.0)
            OT = pp.tile([128, 2, TB], BF16, tag="OT")
            ones_b = pp.tile([128, 64], BF16, tag="onesb")
            nc.vector.memset(ones_b[:], 1.0)

            for _ in range(repeat):
                _body(nc, tc, xp_d, out_d, part_d, rs_d,
                      (wq_sb, wk_sb, wv_sb, wo_sb, bq_sb),
                      xT, QT, KT, Vb, OT, ones_b)
    nc.compile()
    return nc


def _body(nc, tc, xp_d, out_d, part_d, rs_d, ws, xT, QT, KT, Vb, OT, ones_b):
    PSUM = bass.MemorySpace.PSUM
    wq_sb, wk_sb, wv_sb, wo_sb, bq_sb = ws
    groups = [[0, 1, 2, 3], [4, 5, 6, 7]]

    # ---- Phase 1: load xT; project K, Q ([dm, tok]) and V ([tok, dm]) ----
    for j in range(8):
        nc.gpsimd.dma_start(xT[:, j, :], xp_d[:, j, :])
    with (
        tc.tile_pool(name="qkpsum", bufs=4, space=PSUM) as qkp,
        tc.tile_pool(name="vpsum", bufs=2, space=PSUM) as vp,
    ):
        for wsb, dst, biased in ((wk_sb, KT, False), (wq_sb, QT, True)):
            for u in range(2):
                ps = [qkp.tile([128, 512], F32, tag="qk", name="qkps")
                      for _ in range(4)]
                for j in range(8):
                    for t in range(4):
                        nc.tensor.matmul(ps[t][:], wsb[:, j, u, :],
                                         xT[:, j, 512 * t:512 * (t + 1)],
                                         start=(j == 0), stop=(j == 7))
                for t in range(4):
                    if biased:
                        nc.scalar.activation(dst[:, u, 512 * t:512 * (t + 1)],
                                             ps[t][:], Identity,
                                             bias=bq_sb[:, u:u + 1], scale=1.0)
                    else:
                        nc.vector.tensor_copy(dst[:, u, 512 * t:512 * (t + 1)],
                                              ps[t][:])
        for kt in range(16):
            vps = vp.tile([128, HL, 64], F32, tag="vps", name="vps")
            for j in range(8):
                nc.tensor.matmul(vps[:], xT[:, j, 128 * kt:128 * (kt + 1)],
                                 wv_sb[:, j, :], start=(j == 0), stop=(j == 7))
            nc.vector.tensor_copy(Vb[:, kt, :, 0:64], vps[:])

    # ---- Phase 2+3: attention per (q-tile, head) unit, with the softmax
    # normalization software-pipelined one unit behind (so PE never stalls
    # on the reciprocal) and output-projection t-tiles interleaved as soon
    # as their OT columns are normalized; 4-rank bf16 ReduceScatter tail. ----
    with (
        tc.tile_pool(name="pt", bufs=3) as ptp,
        tc.tile_pool(name="spsum", bufs=2, space=PSUM) as sp,
        tc.tile_pool(name="opsum", bufs=2, space=PSUM) as op,
        tc.tile_pool(name="mpsum", bufs=2, space=PSUM) as mp,
        tc.tile_pool(name="nrm", bufs=2) as nrm,
        tc.tile_pool(name="fout", bufs=2) as fo,
    ):
        state = {"norms": 0, "next_t": 0}
        pending = []  # [(o_acc, r_b, a, u, q0)] with recip already emitted

        def norm_recip(o_acc):
            r_b = nrm.tile([128, 512], BF16, tag="rb", name="r_b")
            with nc.allow_low_precision(reason="bf16 softmax recip"):
                nc.vector.reciprocal(r_b[64:65, :], o_acc[64:65, :])
            return r_b

        def norm_finish():
            o_acc, r_b, a, u, q0 = pending.pop(0)
            bc_ps = mp.tile([128, 512], F32, tag="ps512", name="bc_ps",
                            )[0:64, :]
            nc.tensor.matmul(bc_ps[:], ones_b[64:65, :], r_b[64:65, :],
                             start=True, stop=True)
            bc_sb = nrm.tile([64, 512], F32, tag="bcs", name="bc_sb")
            nc.vector.tensor_copy(bc_sb[:], bc_ps[:])
            nc.vector.tensor_mul(OT[64 * a:64 * (a + 1), u, q0:q0 + 512],
                                 o_acc[0:64, :], bc_sb[:])
            state["norms"] += 1

        def oproj_tile(t):
            o_sb = fo.tile([128, D], BF16, tag="fo", name="o_sb")
            for n in range(2):
                acc = mp.tile([128, 512], F32, tag="ps512", name="acc")
                for u in range(2):
                    nc.tensor.matmul(acc[:], OT[:, u, 128 * t:128 * (t + 1)],
                                     wo_sb[:, u, 512 * n:512 * (n + 1)],
                                     start=(u == 0), stop=(u == 1))
                nc.vector.tensor_copy(o_sb[:, 512 * n:512 * (n + 1)], acc[:])
            nc.gpsimd.dma_start(part_d[128 * t:128 * (t + 1), :], o_sb[:])

        def oproj_ready():
            t = state["next_t"]
            return t < 16 and state["norms"] >= 4 * (t // 4) + 4

        for qt in range(4):
            q0 = 512 * qt
            for h in range(HL):
                u, a = divmod(h, 2)
                KT_h = KT[64 * a:64 * (a + 1), u, :]
                QT_h = QT[64 * a:64 * (a + 1), u, :]
                o_acc = op.tile([65, 512], F32, tag="oacc", name="o_acc")
                # PV(kp-1) is emitted after scores(kp): PE (in-order) must
                # never sit behind the EXP it just requested, else ACT runs
                # at ~57% duty (measured 839ns gap between every EXP).
                prev_pt = None
                for kp in range(8):
                    s_ps = sp.tile([128, 1024], F32, tag="sps", name="s_ps")
                    for v in range(2):
                        k0 = 128 * (2 * kp + v)
                        nc.tensor.matmul(s_ps[:, 512 * v:512 * (v + 1)],
                                         KT_h[:, k0:k0 + 128],
                                         QT_h[:, q0:q0 + 512],
                                         start=True, stop=True)
                    pt_t = ptp.tile([128, 1024], BF16, tag="pt", name="pt_t")
                    nc.scalar.activation(pt_t[:], s_ps[:], Exp,
                                         bias=0.0, scale=0.125)
                    if prev_pt is not None:
                        for v in range(2):
                            kc = 2 * (kp - 1) + v
                            nc.tensor.matmul(o_acc[:], Vb[:, kc, h, :],
                                             prev_pt[:, 512 * v:512 * (v + 1)],
                                             start=(kc == 0), stop=False)
                    prev_pt = pt_t
                    if kp == 3 and pending:
                        norm_finish()
                    if kp == 5 and oproj_ready():
                        oproj_tile(state["next_t"])
                        state["next_t"] += 1
                for v in range(2):
                    kc = 14 + v
                    nc.tensor.matmul(o_acc[:], Vb[:, kc, h, :],
                                     prev_pt[:, 512 * v:512 * (v + 1)],
                                     start=False, stop=(kc == 15))
                pending.append((o_acc, norm_recip(o_acc), a, u, q0))
        while pending:
            norm_finish()
        while state["next_t"] < 16:
            oproj_tile(state["next_t"])
            state["next_t"] += 1
        nc.gpsimd.collective_compute(
            "ReduceScatter", mybir.AluOpType.add, replica_groups=groups,
            ins=[part_d[:].opt()], outs=[rs_d[:].opt()])
        nc.gpsimd.dma_start(out_d[:], rs_d[:])


def _in_maps(x, wq, bq, wk, bk, wv, bv, wo, bo):
    maps = []
    for c in range(N_CORES):
        b, g = divmod(c, 4)
        sl = slice(256 * g, 256 * (g + 1))
        xt = np.ascontiguousarray(
            x[b].T.reshape(8, 128, TB).transpose(1, 0, 2)).astype(BF)
        wqp = np.ascontiguousarray(
            wq[:, sl].reshape(8, 128, 2, 128).transpose(1, 0, 2, 3)).astype(BF)
        wkp = np.ascontiguousarray(
            wk[:, sl].reshape(8, 128, 2, 128).transpose(1, 0, 2, 3)).astype(BF)
        wvp = np.ascontiguousarray(
            wv[:, sl].reshape(8, 128, 256).transpose(1, 0, 2)).astype(BF)
        wop = np.ascontiguousarray(
            wo[sl, :].reshape(2, 128, D).transpose(1, 0, 2)).astype(BF)
        bqc = np.ascontiguousarray(bq[sl].reshape(2, 128).T.astype(np.float32))
        maps.append({"xp": xt, "wqp": wqp, "wkp": wkp, "wvp": wvp,
                     "wop": wop, "bqc": bqc})
    return maps


def _make_runner(nc, n_cores):
    """Build the sharded PJRT callable ONCE per nc and reuse it — the
    library path re-creates (and so re-traces/re-lowers) a fresh jax.jit on
    every call, which costs ~1-2s of host time per invocation."""
    import jax
    from jax.sharding import Mesh, PartitionSpec
    from jax.experimental.shard_map import shard_map
    from concourse.bass2jax import (_bass_exec_p, install_neuronx_cc_hook,
                                    partition_id_tensor)

    install_neuronx_cc_hook()
    partition_name = (nc.partition_id_tensor.name
                      if nc.partition_id_tensor else None)
    in_names, out_names, out_avals, zero_outs = [], [], [], []
    for alloc in nc.m.functions[0].allocations:
        if not isinstance(alloc, mybir.MemoryLocationSet):
            continue
        name = alloc.memorylocations[0].name
        if alloc.kind == "ExternalInput":
            if name != partition_name:
                in_names.append(name)
        elif alloc.kind == "ExternalOutput":
            out_names.append(name)
            shape = tuple(alloc.tensor_shape)
            dtype = mybir.dt.np(alloc.dtype)
            out_avals.append(jax.core.ShapedArray(shape, dtype))
            zero_outs.append(np.zeros(shape, dtype))
    n_params = len(in_names)
    all_in_names = list(in_names) + list(out_names)
    if partition_name is not None:
        all_in_names.append(partition_name)
    donate = tuple(range(n_params, n_params + len(out_avals)))

    def _body(*args):
        operands = list(args)
        if partition_name is not None:
            operands.append(partition_id_tensor())
        outs = _bass_exec_p.bind(
            *operands,
            out_avals=tuple(out_avals),
            in_names=tuple(all_in_names),
            out_names=tuple(out_names),
            lowering_input_output_aliases=(),
            sim_require_finite=True,
            sim_require_nnan=True,
            nc=nc,
        )
        return tuple(outs)

    devices = jax.devices()[:n_cores]
    mesh = Mesh(np.asarray(devices), ("core",))
    in_specs = (PartitionSpec("core"),) * (n_params + len(out_avals))
    out_specs = (PartitionSpec("core"),) * len(out_names)
    sharded = jax.jit(
        shard_map(_body, mesh=mesh, in_specs=in_specs, out_specs=out_specs,
                  check_rep=False),
        donate_argnums=donate, keep_unused=True)

    def run(in_maps):
        per_core = [[np.asarray(m[name]) for name in in_names]
                    for m in in_maps]
        concat_in = [np.concatenate([per_core[c][i] for c in range(n_cores)],
                                    axis=0) for i in range(n_params)]
        concat_zeros = [np.zeros((n_cores * z.shape[0], *z.shape[1:]),
                                 z.dtype) for z in zero_outs]
        out_arrs = sharded(*concat_in, *concat_zeros)
        return [
            {name: np.asarray(out_arrs[i]).reshape(n_cores,
                                                   *out_avals[i].shape)[c]
             for i, name in enumerate(out_names)}
            for c in range(n_cores)
        ]

    return run


def _run(nc, maps, cache_key=None):
    if axon_active():
        key = ("runner", cache_key if cache_key is not None else id(nc))
        if key not in _cache:
            _cache[key] = _make_runner(nc, N_CORES)
        return _cache[key](maps)
    res = run_bass_kernel_spmd(nc, maps, core_ids=list(range(N_CORES)),
                               trace=False)
    return res.results


def kernel(**inputs):
    x = np.asarray(inputs["x"], np.float32)
    wq = np.asarray(inputs["wq"], np.float32)
    bq = np.asarray(inputs["bq"], np.float32)
    wk = np.asarray(inputs["wk"], np.float32)
    bk = np.asarray(inputs["bk"], np.float32)
    wv = np.asarray(inputs["wv"], np.float32)
    bv = np.asarray(inputs["bv"], np.float32)
    wo = np.asarray(inputs["wo"], np.float32)
    bo = np.asarray(inputs["bo"], np.float32)
    maps = _in_maps(x, wq, bq, wk, bk, wv, bv, wo, bo)
    if "nc" not in _cache:
        _cache["nc"] = _build()
    results = _run(_cache["nc"], maps, cache_key="main")
    out = np.concatenate(
        [np.asarray(results[c]["out"], np.float32) for c in range(N_CORES)],
        axis=0)
    bo_eff = bo + bv.astype(np.float32) @ wo
    return (out + bo_eff.reshape(1, D)).reshape(B, S, D).astype(np.float32)


# revision 40
# speedup vs baseline: 1.0374x; 1.0374x over previous
"""MultiHeadAttention on 8 TRN2 NeuronCores — v5.

Sharding: core c = (batch b=c//4, head-group g=c%4). Each core computes
Q/K/V projections for its 4 heads (d_model slice 256g:256g+256) over its
batch's 2048 tokens, full attention for those heads, a partial output
projection over all 2048 batch tokens (contraction over its 256 dm rows),
then ONE 4-rank bf16 ReduceScatter (4MB/rank) hands each core its final
512-token output slice.

vs v3: no x AllGather, RS shrinks 16MB f32 8-rank -> 4MB bf16 4-rank,
all-bf16 matmuls, V projected directly in [tok, dm] layout (no PE
transposes), bk dropped (softmax is invariant to per-query score
shifts), bv folded into bo on the host (bo_eff = bo + bv @ wo).
"""

import numpy as np
import ml_dtypes

import concourse.bass as bass
import concourse.tile as tile
from concourse import bacc, mybir
from concourse.bass_utils import axon_active, run_bass_kernel_spmd

N_CORES = 8
B, S, D = 2, 2048, 1024
H = 16          # heads total
HL = 4          # heads per core
DK = 64
TB = 2048       # tokens per batch (local q/k length)
TOUT = 512      # output tokens per core
F32 = mybir.dt.float32
BF16 = mybir.dt.bfloat16
Exp = mybir.ActivationFunctionType.Exp
Identity = mybir.ActivationFunctionType.Identity
Reciprocal = mybir.ActivationFunctionType.Reciprocal
BF = ml_dtypes.bfloat16

_cache = {}


def _build(repeat=1):
    nc = bacc.Bacc("TRN2", target_bir_lowering=False, debug=False,
                   num_devices=N_CORES)
    xp_d = nc.dram_tensor("xp", [128, 8, TB], BF16, kind="ExternalInput").ap()
    wq_d = nc.dram_tensor("wqp", [128, 8, 2, 128], BF16, kind="ExternalInput").ap()
    wk_d = nc.dram_tensor("wkp", [128, 8, 2, 128], BF16, kind="ExternalInput").ap()
    wv_d = nc.dram_tensor("wvp", [128, 8, 256], BF16, kind="ExternalInput").ap()
    wo_d = nc.dram_tensor("wop", [128, 2, D], BF16, kind="ExternalInput").ap()
    bq_d = nc.dram_tensor("bqc", [128, 2], F32, kind="ExternalInput").ap()
    out_d = nc.dram_tensor("out", [TOUT, D], BF16, kind="ExternalOutput").ap()

    with tile.TileContext(nc) as tc:
        with (
            tc.tile_pool(name="dram", bufs=1, space="DRAM") as dram,
            tc.tile_pool(name="persist", bufs=1) as pp,
        ):
            part_d = dram.tile([TB, D], BF16, tag="partd")
            rs_d = dram.tile([TOUT, D], BF16, tag="rsd")

            wq_sb = pp.tile([128, 8, 2, 128], BF16, tag="wq")
            wk_sb = pp.tile([128, 8, 2, 128], BF16, tag="wk")
            wv_sb = pp.tile([128, 8, 256], BF16, tag="wv")
            wo_sb = pp.tile([128, 2, D], BF16, tag="wo")
            bq_sb = pp.tile([128, 2], F32, tag="bq")
            nc.gpsimd.dma_start(wq_sb[:], wq_d[:])
            nc.gpsimd.dma_start(wk_sb[:], wk_d[:])
            nc.gpsimd.dma_start(wv_sb[:], wv_d[:])
            nc.gpsimd.dma_start(wo_sb[:], wo_d[:])
            nc.gpsimd.dma_start(bq_sb[:], bq_d[:])

            xT = pp.tile([128, 8, TB], BF16, tag="xT")
            QT = pp.tile([128, 2, TB], BF16, tag="QT")
            KT = pp.tile([128, 2, TB], BF16, tag="KT")
            # V in natural [tok, dm] layout + ones column for the softmax
            # denominator: lhsT chunks [128 tok, 64 V | 1].
            Vb = pp.tile([128, 16, HL, 65], BF16, tag="vb")
            nc.vector.memset(Vb[:, :, :, 64:65], 1.0)
            OT = pp.tile([128, 2, TB], BF16, tag="OT")
            ones_b = pp.tile([128, 64], BF16, tag="onesb")
            nc.vector.memset(ones_b[:], 1.0)

            for _ in range(repeat):
                _body(nc, tc, xp_d, out_d, part_d, rs_d,
                      (wq_sb, wk_sb, wv_sb, wo_sb, bq_sb),
                      xT, QT, KT, Vb, OT, ones_b)
    nc.compile()
    return nc


def _body(nc, tc, xp_d, out_d, part_d, rs_d, ws, xT, QT, KT, Vb, OT, ones_b):
    PSUM = bass.MemorySpace.PSUM
    wq_sb, wk_sb, wv_sb, wo_sb, bq_sb = ws
    groups = [[0, 1, 2, 3], [4, 5, 6, 7]]

    # ---- Phase 1: load xT; project K, Q ([dm, tok]) and V ([tok, dm]) ----
    for j in range(8):
        nc.gpsimd.dma_start(xT[:, j, :], xp_d[:, j, :])
    with (
        tc.tile_pool(name="qkpsum", bufs=4, space=PSUM) as qkp,
        tc.tile_pool(name="vpsum", bufs=2, space=PSUM) as vp,
    ):
        for wsb, dst, biased in ((wk_sb, KT, False), (wq_sb, QT, True)):
            for u in range(2):
                ps = [qkp.tile([128, 512], F32, tag="qk", name="qkps")
                      for _ in range(4)]
                for j in range(8):
                    for t in range(4):
                        nc.tensor.matmul(ps[t][:], wsb[:, j, u, :],
                                         xT[:, j, 512 * t:512 * (t + 1)],
                                         start=(j == 0), stop=(j == 7))
                for t in range(4):
                    if biased:
                        nc.scalar.activation(dst[:, u, 512 * t:512 * (t + 1)],
                                             ps[t][:], Identity,
                                             bias=bq_sb[:, u:u + 1], scale=1.0)
                    else:
                        nc.vector.tensor_copy(dst[:, u, 512 * t:512 * (t + 1)],
                                              ps[t][:])
        for kt in range(16):
            vps = vp.tile([128, HL, 64], F32, tag="vps", name="vps")
            for j in range(8):
                nc.tensor.matmul(vps[:], xT[:, j, 128 * kt:128 * (kt + 1)],
                                 wv_sb[:, j, :], start=(j == 0), stop=(j == 7))
            nc.vector.tensor_copy(Vb[:, kt, :, 0:64], vps[:])

    # ---- Phase 2+3: attention per (q-tile, head) unit, with the softmax
    # normalization software-pipelined one unit behind (so PE never stalls
    # on the reciprocal) and output-projection t-tiles interleaved as soon
    # as their OT columns are normalized; 4-rank bf16 ReduceScatter tail. ----
    with (
        tc.tile_pool(name="pt", bufs=4) as ptp,
        tc.tile_pool(name="spsum", bufs=2, space=PSUM) as sp,
        tc.tile_pool(name="opsum", bufs=2, space=PSUM) as op,
        tc.tile_pool(name="mpsum", bufs=2, space=PSUM) as mp,
        tc.tile_pool(name="nrm", bufs=2) as nrm,
        tc.tile_pool(name="fout", bufs=2) as fo,
    ):
        state = {"norms": 0, "next_t": 0}
        pending = []  # [(o_acc, r_b, a, u, q0)] with recip already emitted

        def norm_recip(o_acc):
            r_b = nrm.tile([128, 512], BF16, tag="rb", name="r_b")
            with nc.allow_low_precision(reason="bf16 softmax recip"):
                nc.vector.reciprocal(r_b[64:65, :], o_acc[64:65, :])
            return r_b

        def norm_finish():
            o_acc, r_b, a, u, q0 = pending.pop(0)
            bc_ps = mp.tile([128, 512], F32, tag="ps512", name="bc_ps",
                            )[0:64, :]
            nc.tensor.matmul(bc_ps[:], ones_b[64:65, :], r_b[64:65, :],
                             start=True, stop=True)
            bc_sb = nrm.tile([64, 512], F32, tag="bcs", name="bc_sb")
            nc.vector.tensor_copy(bc_sb[:], bc_ps[:])
            nc.vector.tensor_mul(OT[64 * a:64 * (a + 1), u, q0:q0 + 512],
                                 o_acc[0:64, :], bc_sb[:])
            state["norms"] += 1

        def oproj_tile(t):
            o_sb = fo.tile([128, D], BF16, tag="fo", name="o_sb")
            for n in range(2):
                acc = mp.tile([128, 512], F32, tag="ps512", name="acc")
                for u in range(2):
                    nc.tensor.matmul(acc[:], OT[:, u, 128 * t:128 * (t + 1)],
                                     wo_sb[:, u, 512 * n:512 * (n + 1)],
                                     start=(u == 0), stop=(u == 1))
                nc.vector.tensor_copy(o_sb[:, 512 * n:512 * (n + 1)], acc[:])
            nc.gpsimd.dma_start(part_d[128 * t:128 * (t + 1), :], o_sb[:])

        def oproj_ready():
            t = state["next_t"]
            return t < 16 and state["norms"] >= 4 * (t // 4) + 4

        for qt in range(4):
            q0 = 512 * qt
            for h in range(HL):
                u, a = divmod(h, 2)
                KT_h = KT[64 * a:64 * (a + 1), u, :]
                QT_h = QT[64 * a:64 * (a + 1), u, :]
                o_acc = op.tile([65, 512], F32, tag="oacc", name="o_acc")
                # PV(kp-1) is emitted after scores(kp): PE (in-order) must
                # never sit behind the EXP it just requested, else ACT runs
                # at ~57% duty (measured 839ns gap between every EXP).
                prev_pt = None
                for kp in range(8):
                    s_ps = sp.tile([128, 1024], F32, tag="sps", name="s_ps")
                    for v in range(2):
                        k0 = 128 * (2 * kp + v)
                        nc.tensor.matmul(s_ps[:, 512 * v:512 * (v + 1)],
                                         KT_h[:, k0:k0 + 128],
                                         QT_h[:, q0:q0 + 512],
                                         start=True, stop=True)
                    pt_t = ptp.tile([128, 1024], BF16, tag="pt", name="pt_t")
                    nc.scalar.activation(pt_t[:], s_ps[:], Exp,
                                         bias=0.0, scale=0.125)
                    if prev_pt is not None:
                        for v in range(2):
                            kc = 2 * (kp - 1) + v
                            nc.tensor.matmul(o_acc[:], Vb[:, kc, h, :],
                                             prev_pt[:, 512 * v:512 * (v + 1)],
                                             start=(kc == 0), stop=False)
                    prev_pt = pt_t
                    if kp == 3 and pending:
                        norm_finish()
                    if kp == 5 and oproj_ready():
                        oproj_tile(state["next_t"])
                        state["next_t"] += 1
                for v in range(2):
                    kc = 14 + v
                    nc.tensor.matmul(o_acc[:], Vb[:, kc, h, :],
                                     prev_pt[:, 512 * v:512 * (v + 1)],
                                     start=False, stop=(kc == 15))
                pending.append((o_acc, norm_recip(o_acc), a, u, q0))
        while pending:
            norm_finish()
        while state["next_t"] < 16:
            oproj_tile(state["next_t"])
            state["next_t"] += 1
        nc.gpsimd.collective_compute(
            "ReduceScatter", mybir.AluOpType.add, replica_groups=groups,
            ins=[part_d[:].opt()], outs=[rs_d[:].opt()])
        nc.gpsimd.dma_start(out_d[:], rs_d[:])


def _in_maps(x, wq, bq, wk, bk, wv, bv, wo, bo):
    maps = []
    for c in range(N_CORES):
        b, g = divmod(c, 4)
        sl = slice(256 * g, 256 * (g + 1))
        xt = np.ascontiguousarray(
            x[b].T.reshape(8, 128, TB).transpose(1, 0, 2)).astype(BF)
        wqp = np.ascontiguousarray(
            wq[:, sl].reshape(8, 128, 2, 128).transpose(1, 0, 2, 3)).astype(BF)
        wkp = np.ascontiguousarray(
            wk[:, sl].reshape(8, 128, 2, 128).transpose(1, 0, 2, 3)).astype(BF)
        wvp = np.ascontiguousarray(
            wv[:, sl].reshape(8, 128, 256).transpose(1, 0, 2)).astype(BF)
        wop = np.ascontiguousarray(
            wo[sl, :].reshape(2, 128, D).transpose(1, 0, 2)).astype(BF)
        bqc = np.ascontiguousarray(bq[sl].reshape(2, 128).T.astype(np.float32))
        maps.append({"xp": xt, "wqp": wqp, "wkp": wkp, "wvp": wvp,
                     "wop": wop, "bqc": bqc})
    return maps


def _make_runner(nc, n_cores):
    """Build the sharded PJRT callable ONCE per nc and reuse it — the
    library path re-creates (and so re-traces/re-lowers) a fresh jax.jit on
    every call, which costs ~1-2s of host time per invocation."""
    import jax
    from jax.sharding import Mesh, PartitionSpec
    from jax.experimental.shard_map import shard_map
    from concourse.bass2jax import (_bass_exec_p, install_neuronx_cc_hook,
                                    partition_id_tensor)

    install_neuronx_cc_hook()
    partition_name = (nc.partition_id_tensor.name
                      if nc.partition_id_tensor else None)
    in_names, out_names, out_avals, zero_outs = [], [], [], []
    for alloc in nc.m.functions[0].allocations:
        if not isinstance(alloc, mybir.MemoryLocationSet):
            continue
        name = alloc.memorylocations[0].name
        if alloc.kind == "ExternalInput":
            if name != partition_name:
                in_names.append(name)
        elif alloc.kind == "ExternalOutput":
            out_names.append(name)
            shape = tuple(alloc.tensor_shape)
            dtype = mybir.dt.np(alloc.dtype)
            out_avals.append(jax.core.ShapedArray(shape, dtype))
            zero_outs.append(np.zeros(shape, dtype))
    n_params = len(in_names)
    all_in_names = list(in_names) + list(out_names)
    if partition_name is not None:
        all_in_names.append(partition_name)
    donate = tuple(range(n_params, n_params + len(out_avals)))

    def _body(*args):
        operands = list(args)
        if partition_name is not None:
            operands.append(partition_id_tensor())
        outs = _bass_exec_p.bind(
            *operands,
            out_avals=tuple(out_avals),
            in_names=tuple(all_in_names),
            out_names=tuple(out_names),
            lowering_input_output_aliases=(),
            sim_require_finite=True,
            sim_require_nnan=True,
            nc=nc,
        )
        return tuple(outs)

    devices = jax.devices()[:n_cores]
    mesh = Mesh(np.asarray(devices), ("core",))
    in_specs = (PartitionSpec("core"),) * (n_params + len(out_avals))
    out_specs = (PartitionSpec("core"),) * len(out_names)
    sharded = jax.jit(
        shard_map(_body, mesh=mesh, in_specs=in_specs, out_specs=out_specs,
                  check_rep=False),
        donate_argnums=donate, keep_unused=True)

    def run(in_maps):
        per_core = [[np.asarray(m[name]) for name in in_names]
                    for m in in_maps]
        concat_in = [np.concatenate([per_core[c][i] for c in range(n_cores)],
                                    axis=0) for i in range(n_params)]
        concat_zeros = [np.zeros((n_cores * z.shape[0], *z.shape[1:]),
                                 z.dtype) for z in zero_outs]
        out_arrs = sharded(*concat_in, *concat_zeros)
        return [
            {name: np.asarray(out_arrs[i]).reshape(n_cores,
                                                   *out_avals[i].shape)[c]
             for i, name in enumerate(out_names)}
            for c in range(n_cores)
        ]

    return run


def _run(nc, maps, cache_key=None):
    if axon_active():
        key = ("runner", cache_key if cache_key is not None else id(nc))
        if key not in _cache:
            _cache[key] = _make_runner(nc, N_CORES)
        return _cache[key](maps)
    res = run_bass_kernel_spmd(nc, maps, core_ids=list(range(N_CORES)),
                               trace=False)
    return res.results


def kernel(**inputs):
    x = np.asarray(inputs["x"], np.float32)
    wq = np.asarray(inputs["wq"], np.float32)
    bq = np.asarray(inputs["bq"], np.float32)
    wk = np.asarray(inputs["wk"], np.float32)
    bk = np.asarray(inputs["bk"], np.float32)
    wv = np.asarray(inputs["wv"], np.float32)
    bv = np.asarray(inputs["bv"], np.float32)
    wo = np.asarray(inputs["wo"], np.float32)
    bo = np.asarray(inputs["bo"], np.float32)
    maps = _in_maps(x, wq, bq, wk, bk, wv, bv, wo, bo)
    if "nc" not in _cache:
        _cache["nc"] = _build()
    results = _run(_cache["nc"], maps, cache_key="main")
    out = np.concatenate(
        [np.asarray(results[c]["out"], np.float32) for c in range(N_CORES)],
        axis=0)
    bo_eff = bo + bv.astype(np.float32) @ wo
    return (out + bo_eff.reshape(1, D)).reshape(B, S, D).astype(np.float32)
